# revision 29
# baseline (speedup 1.0000x reference)
"""WaveNet-style gated dilated conv layer on 8 Trainium2 NeuronCores.

Strategy: data-parallel over batch (B=8 -> 1 batch element per core).
Per core (batch b):
  z_tanh = sum_k Wc_tanh[k] @ x[:, t-d*(2-k)] + Wcond_tanh @ cond + bias
  z_sig  = likewise for the second half of the 2R conv channels
  h      = tanh(z_tanh) * sigmoid(z_sig)
  out    = W_out @ h, skip = W_skip @ h  (1x1 convs)
All matmuls run in bf16 with fp32 PSUM accumulation.  x and cond are cast
to bf16 on host to halve HBM->SBUF traffic; x is also causal-padded on
host so no on-chip memset is needed.  z biases fold into the activation
instruction; out/skip biases (zero-filled per spec) are added on host
only if nonzero.

TRN2 matmul instructions only have room for a single semaphore wait, so
the kernel is structured so no matmul ever needs two: input DMAs are
"observed" by the PE via standalone ldweights instructions before the
first matmul that would otherwise combine a DMA wait with a PSUM WAR
wait.
"""

import sys

for _p in ("/opt/trn_rl_repo",):
    if _p not in sys.path:
        sys.path.append(_p)

from contextlib import ExitStack

import ml_dtypes
import numpy as np

import concourse.bacc as bacc
import concourse.bass as bass
import concourse.tile as tile
from concourse import mybir
from concourse.bass_utils import run_bass_kernel_spmd

B, CIN, T = 8, 128, 16384
R, S, CC, KW = 128, 128, 80, 3
NT = 512           # time-tile width (one PSUM bank of fp32)
NTILES = T // NT
NCHUNKS = 8        # DMA chunks for the big input loads
N_CORES = 8

BF16 = mybir.dt.bfloat16
FP32 = mybir.dt.float32
AF = mybir.ActivationFunctionType

_built = {}
_TRACE = False        # set True (e.g. by a test harness) to capture an NTFF profile
_last_results = None  # BassKernelResults of the most recent run


# Streaming chunk widths: small at the head (fast first-compute), large in
# the middle (few DMA triggers), small at the tail (fast final drain).
CHUNK_WIDTHS = [512, 512, 1024] + [2048] * 6 + [1536, 512]
assert sum(CHUNK_WIDTHS) == T
CHUNK_STARTS = [sum(CHUNK_WIDTHS[:i]) for i in range(len(CHUNK_WIDTHS))]
NCH = len(CHUNK_WIDTHS)
PREFETCH = 2         # chunk lookahead beyond the current group


def _build(dilation: int) -> bass.Bass:
    pad = dilation * (KW - 1)

    nc = bacc.Bacc("TRN2", target_bir_lowering=False, debug=False, num_devices=N_CORES)

    x = nc.declare_dram_parameter("x", [CIN, pad + T], BF16, isOutput=False)
    cond = nc.declare_dram_parameter("cond", [CC, T], BF16, isOutput=False)
    # packed lhsT weights (already transposed to [Cin, Cout] on host)
    wconv = nc.declare_dram_parameter("wconv", [CIN, 2 * KW * R], BF16, isOutput=False)
    wcond = nc.declare_dram_parameter("wcond", [CC, 2 * R], BF16, isOutput=False)
    wos = nc.declare_dram_parameter("wos", [R, R + S], BF16, isOutput=False)
    zbias = nc.declare_dram_parameter("zbias", [R, 2], FP32, isOutput=False)

    out = nc.declare_dram_parameter("out", [R, T], FP32, isOutput=True)
    skip = nc.declare_dram_parameter("skip", [S, T], FP32, isOutput=True)

    with tile.TileContext(nc) as tc, ExitStack() as ctx:
        consts = ctx.enter_context(tc.tile_pool(name="consts", bufs=1))
        inpool = ctx.enter_context(tc.tile_pool(name="inp", bufs=PREFETCH + 2))
        hpool = ctx.enter_context(tc.tile_pool(name="h", bufs=3))
        opool = ctx.enter_context(tc.tile_pool(name="o", bufs=3))
        zpsum = ctx.enter_context(tc.tile_pool(name="zpsum", bufs=2, space="PSUM"))
        opsum = ctx.enter_context(tc.tile_pool(name="opsum", bufs=2, space="PSUM"))

        # chunk 0 is loaded before anything else: every HWDGE trigger costs
        # ~0.6us of serial sequencer time, so the first-needed data goes first
        xc_tiles = [None] * NCH
        cc_tiles = [None] * NCH

        def load_chunk(g):
            gs, gw = CHUNK_STARTS[g], CHUNK_WIDTHS[g]
            xc = inpool.tile([CIN, pad + gw], BF16, tag="xc")
            nc.sync.dma_start(xc[:], x[:, gs : gs + pad + gw])
            cc = inpool.tile([CC, gw], BF16, tag="cc")
            nc.sync.dma_start(cc[:], cond[:, gs : gs + gw])
            xc_tiles[g], cc_tiles[g] = xc, cc

        zbias_sb = consts.tile([R, 2], FP32)
        nc.sync.dma_start(zbias_sb[:], zbias[:])
        load_chunk(0)
        wconv_sb = consts.tile([CIN, 2 * KW * R], BF16)
        nc.sync.dma_start(wconv_sb[:], wconv[:])
        wcond_sb = consts.tile([CC, 2 * R], BF16)
        nc.sync.dma_start(wcond_sb[:], wcond[:])
        load_chunk(1)
        wos_sb = consts.tile([R, R + S], BF16)
        nc.sync.dma_start(wos_sb[:], wos[:])

        # Warm-up during the input-load head: ~9 matmuls on uninitialized
        # SBUF kick the PE HAM to 8/8 before real work arrives, and two
        # 1-column activations trigger the tanh/sigmoid table load (~2.7us).
        garbage = consts.tile([CIN, NT], BF16)
        act_sink = consts.tile([R, 1], FP32)
        nc.vector.memset(garbage[:], 0.0)
        nc.vector.memset(act_sink[:], 0.0)
        for _ in range(14):
            wz = zpsum.tile([R, NT], FP32, tag="ztan")
            nc.tensor.matmul(wz[:], garbage[:, 0:R], garbage[:], start=True, stop=True)
        nc.scalar.activation(act_sink[:], act_sink[:], AF.Tanh, bias=zbias_sb[:, 0:1])
        nc.scalar.activation(act_sink[:], act_sink[:], AF.Sigmoid, bias=zbias_sb[:, 1:2])

        # Output DMA triggers are deferred by one chunk group so their waits
        # (on the staging copies) are satisfied when the SP sequencer reaches
        # them — an unsatisfied wait would head-of-line block the input
        # triggers queued behind them on the same HWDGE ring.
        pending_out = []

        def flush_out():
            while pending_out:
                gs, gw, osb, ssb = pending_out.pop()
                nc.sync.dma_start(out[:, gs : gs + gw], osb[:])
                nc.sync.dma_start(skip[:, gs : gs + gw], ssb[:])

        for g in range(NCH):
            gs, gw = CHUNK_STARTS[g], CHUNK_WIDTHS[g]
            for gg in range(g + 1, min(g + PREFETCH + 1, NCH)):
                if xc_tiles[gg] is None:
                    load_chunk(gg)
            xc, cc = xc_tiles[g], cc_tiles[g]
            # let PE observe the chunk DMA sems on a standalone ldweights
            # so no accumulating matmul needs two waits
            nc.tensor.ldweights(xc[:, 0:R])
            nc.tensor.ldweights(cc[:, 0:R])

            osb = opool.tile([R, gw], FP32, tag="osb")
            ssb = opool.tile([S, gw], FP32, tag="ssb")
            for l0 in range(0, gw, NT):
                w = min(NT, gw - l0)
                ztan = zpsum.tile([R, w], FP32, tag="ztan")
                zsig = zpsum.tile([R, w], FP32, tag="zsig")
                for k in range(KW):
                    xs = xc[:, l0 + dilation * k : l0 + dilation * k + w]
                    nc.tensor.matmul(
                        ztan[:], wconv_sb[:, k * R : (k + 1) * R], xs,
                        start=(k == 0), stop=False,
                    )
                nc.tensor.matmul(
                    ztan[:], wcond_sb[:, 0:R], cc[:, l0 : l0 + w],
                    start=False, stop=True,
                )
                for k in range(KW):
                    xs = xc[:, l0 + dilation * k : l0 + dilation * k + w]
                    nc.tensor.matmul(
                        zsig[:], wconv_sb[:, (KW + k) * R : (KW + k + 1) * R], xs,
                        start=(k == 0), stop=False,
                    )
                nc.tensor.matmul(
                    zsig[:], wcond_sb[:, R : 2 * R], cc[:, l0 : l0 + w],
                    start=False, stop=True,
                )

                th = hpool.tile([R, w], BF16, tag="th")
                nc.scalar.activation(th[:], ztan[:], AF.Tanh, bias=zbias_sb[:, 0:1])
                sg = hpool.tile([R, w], BF16, tag="sg")
                nc.scalar.activation(sg[:], zsig[:], AF.Sigmoid, bias=zbias_sb[:, 1:2])
                h = hpool.tile([R, w], BF16, tag="h")
                nc.vector.tensor_mul(h[:], th[:], sg[:])

                for o in range(0, w, NT):
                    po = opsum.tile([R, NT], FP32, tag="po")
                    nc.tensor.matmul(
                        po[:], wos_sb[:, 0:R], h[:, o : o + NT], start=True, stop=True
                    )
                    ps = opsum.tile([S, NT], FP32, tag="ps")
                    nc.tensor.matmul(
                        ps[:], wos_sb[:, R : R + S], h[:, o : o + NT],
                        start=True, stop=True,
                    )
                    nc.vector.tensor_copy(osb[:, l0 + o : l0 + o + NT], po[:])
                    nc.vector.tensor_copy(ssb[:, l0 + o : l0 + o + NT], ps[:])

            flush_out()
            if g == NCH - 1:
                # no input triggers remain behind these: flush immediately
                nc.sync.dma_start(out[:, gs : gs + gw], osb[:])
                nc.sync.dma_start(skip[:, gs : gs + gw], ssb[:])
            else:
                pending_out.append((gs, gw, osb, ssb))

    nc.compile()
    return nc


def _pack_weights(w_conv, w_cond, w_out, w_skip, b_conv, b_cond):
    bf = ml_dtypes.bfloat16
    wconv_p = np.empty((CIN, 2 * KW * R), dtype=bf)
    for k in range(KW):
        wconv_p[:, k * R : (k + 1) * R] = w_conv[0:R, :, k].T.astype(bf)
        wconv_p[:, (KW + k) * R : (KW + k + 1) * R] = w_conv[R : 2 * R, :, k].T.astype(bf)
    wcond_p = np.concatenate(
        [w_cond[0:R, :, 0].T, w_cond[R : 2 * R, :, 0].T], axis=1
    ).astype(bf)
    wos_p = np.concatenate([w_out[:, :, 0].T, w_skip[:, :, 0].T], axis=1).astype(bf)
    zbias_p = np.stack(
        [b_conv[:R] + b_cond[:R], b_conv[R:] + b_cond[R:]], axis=1
    ).astype(np.float32)
    return wconv_p, wcond_p, wos_p, zbias_p


def kernel(**inputs):
    x = np.asarray(inputs["x"], dtype=np.float32)
    cond = np.asarray(inputs["cond"], dtype=np.float32)
    w_conv = np.asarray(inputs["w_conv"], dtype=np.float32)
    b_conv = np.asarray(inputs["b_conv"], dtype=np.float32)
    w_cond = np.asarray(inputs["w_cond"], dtype=np.float32)
    b_cond = np.asarray(inputs["b_cond"], dtype=np.float32)
    w_out = np.asarray(inputs["w_out"], dtype=np.float32)
    b_out = np.asarray(inputs["b_out"], dtype=np.float32)
    w_skip = np.asarray(inputs["w_skip"], dtype=np.float32)
    b_skip = np.asarray(inputs["b_skip"], dtype=np.float32)
    dilation = int(np.asarray(inputs["dilation"]))
    pad = dilation * (KW - 1)

    if dilation not in _built:
        _built[dilation] = _build(dilation)
    nc = _built[dilation]

    wconv_p, wcond_p, wos_p, zbias_p = _pack_weights(
        w_conv, w_cond, w_out, w_skip, b_conv, b_cond
    )
    bf = ml_dtypes.bfloat16
    xb = np.zeros((B, CIN, pad + T), dtype=bf)
    xb[:, :, pad:] = x.astype(bf)
    cb = np.ascontiguousarray(cond.astype(bf))

    in_maps = [
        {
            "x": xb[b],
            "cond": cb[b],
            "wconv": wconv_p,
            "wcond": wcond_p,
            "wos": wos_p,
            "zbias": zbias_p,
        }
        for b in range(B)
    ]
    br = run_bass_kernel_spmd(nc, in_maps, list(range(N_CORES)), trace=_TRACE)
    global _last_results
    _last_results = br
    res = br.results
    output = np.stack([res[b]["out"] for b in range(B)])
    skip = np.stack([res[b]["skip"] for b in range(B)])
    if b_out.any():
        output = output + b_out[None, :, None]
    if b_skip.any():
        skip = skip + b_skip[None, :, None]
    return (output, skip)


# revision 30
# speedup vs baseline: 1.0332x; 1.0332x over previous
"""WaveNet-style gated dilated conv layer on 8 Trainium2 NeuronCores.

Strategy: data-parallel over batch (B=8 -> 1 batch element per core).
Per core (batch b):
  z_tanh = sum_k Wc_tanh[k] @ x[:, t-d*(2-k)] + Wcond_tanh @ cond + bias
  z_sig  = likewise for the second half of the 2R conv channels
  h      = tanh(z_tanh) * sigmoid(z_sig)
  out    = W_out @ h, skip = W_skip @ h  (1x1 convs)
All matmuls run in bf16 with fp32 PSUM accumulation.  x and cond are cast
to bf16 on host to halve HBM->SBUF traffic; x is also causal-padded on
host so no on-chip memset is needed.  z biases fold into the activation
instruction; out/skip biases (zero-filled per spec) are added on host
only if nonzero.

TRN2 matmul instructions only have room for a single semaphore wait, so
the kernel is structured so no matmul ever needs two: input DMAs are
"observed" by the PE via standalone ldweights instructions before the
first matmul that would otherwise combine a DMA wait with a PSUM WAR
wait.
"""

import sys

for _p in ("/opt/trn_rl_repo",):
    if _p not in sys.path:
        sys.path.append(_p)

from contextlib import ExitStack

import ml_dtypes
import numpy as np

import concourse.bacc as bacc
import concourse.bass as bass
import concourse.tile as tile
from concourse import mybir
from concourse.bass_utils import run_bass_kernel_spmd

B, CIN, T = 8, 128, 16384
R, S, CC, KW = 128, 128, 80, 3
NT = 512           # time-tile width (one PSUM bank of fp32)
NTILES = T // NT
NCHUNKS = 8        # DMA chunks for the big input loads
N_CORES = 8

BF16 = mybir.dt.bfloat16
FP32 = mybir.dt.float32
AF = mybir.ActivationFunctionType

_built = {}
_TRACE = False        # set True (e.g. by a test harness) to capture an NTFF profile
_last_results = None  # BassKernelResults of the most recent run


# Streaming chunk widths: small at the head (fast first-compute), large in
# the middle (few DMA triggers), small at the tail (fast final drain).
CHUNK_WIDTHS = [512, 1536] + [2048] * 6 + [1024, 1024]
assert sum(CHUNK_WIDTHS) == T
CHUNK_STARTS = [sum(CHUNK_WIDTHS[:i]) for i in range(len(CHUNK_WIDTHS))]
NCH = len(CHUNK_WIDTHS)
PREFETCH = 2         # chunk lookahead beyond the current group


def _build(dilation: int) -> bass.Bass:
    pad = dilation * (KW - 1)

    nc = bacc.Bacc("TRN2", target_bir_lowering=False, debug=False, num_devices=N_CORES)

    x = nc.declare_dram_parameter("x", [CIN, pad + T], BF16, isOutput=False)
    cond = nc.declare_dram_parameter("cond", [CC, T], BF16, isOutput=False)
    # packed lhsT weights (already transposed to [Cin, Cout] on host)
    wconv = nc.declare_dram_parameter("wconv", [CIN, 2 * KW * R], BF16, isOutput=False)
    wcond = nc.declare_dram_parameter("wcond", [CC, 2 * R], BF16, isOutput=False)
    wos = nc.declare_dram_parameter("wos", [R, R + S], BF16, isOutput=False)
    zbias = nc.declare_dram_parameter("zbias", [R, 2], FP32, isOutput=False)

    out = nc.declare_dram_parameter("out", [R, T], FP32, isOutput=True)
    skip = nc.declare_dram_parameter("skip", [S, T], FP32, isOutput=True)

    with tile.TileContext(nc) as tc, ExitStack() as ctx:
        consts = ctx.enter_context(tc.tile_pool(name="consts", bufs=1))
        inpool = ctx.enter_context(tc.tile_pool(name="inp", bufs=PREFETCH + 2))
        hpool = ctx.enter_context(tc.tile_pool(name="h", bufs=3))
        opool = ctx.enter_context(tc.tile_pool(name="o", bufs=3))
        zpsum = ctx.enter_context(tc.tile_pool(name="zpsum", bufs=2, space="PSUM"))
        opsum = ctx.enter_context(tc.tile_pool(name="opsum", bufs=2, space="PSUM"))

        # chunk 0 is loaded before anything else: every HWDGE trigger costs
        # ~0.6us of serial sequencer time, so the first-needed data goes first
        xc_tiles = [None] * NCH
        cc_tiles = [None] * NCH

        def load_chunk(g):
            gs, gw = CHUNK_STARTS[g], CHUNK_WIDTHS[g]
            xc = inpool.tile([CIN, pad + gw], BF16, tag="xc")
            nc.sync.dma_start(xc[:], x[:, gs : gs + pad + gw])
            cc = inpool.tile([CC, gw], BF16, tag="cc")
            nc.sync.dma_start(cc[:], cond[:, gs : gs + gw])
            xc_tiles[g], cc_tiles[g] = xc, cc

        load_chunk(0)
        wconv_sb = consts.tile([CIN, 2 * KW * R], BF16)
        nc.sync.dma_start(wconv_sb[:], wconv[:])
        wcond_sb = consts.tile([CC, 2 * R], BF16)
        nc.sync.dma_start(wcond_sb[:], wcond[:])
        wos_sb = consts.tile([R, R + S], BF16)
        nc.sync.dma_start(wos_sb[:], wos[:])
        zbias_sb = consts.tile([R, 2], FP32)
        nc.sync.dma_start(zbias_sb[:], zbias[:])
        load_chunk(1)

        # Warm-up during the input-load head: ~9 matmuls on uninitialized
        # SBUF kick the PE HAM to 8/8 before real work arrives, and two
        # 1-column activations trigger the tanh/sigmoid table load (~2.7us).
        garbage = consts.tile([CIN, NT], BF16)
        act_sink = consts.tile([R, 1], FP32)
        nc.vector.memset(garbage[:], 0.0)
        nc.vector.memset(act_sink[:], 0.0)
        for _ in range(9):
            wz = zpsum.tile([R, NT], FP32, tag="ztan")
            nc.tensor.matmul(wz[:], garbage[:, 0:R], garbage[:], start=True, stop=True)
        nc.scalar.activation(act_sink[:], act_sink[:], AF.Tanh, bias=zbias_sb[:, 0:1])
        nc.scalar.activation(act_sink[:], act_sink[:], AF.Sigmoid, bias=zbias_sb[:, 1:2])

        # Output DMA triggers are deferred by one chunk group so their waits
        # (on the staging copies) are satisfied when the SP sequencer reaches
        # them — an unsatisfied wait would head-of-line block the input
        # triggers queued behind them on the same HWDGE ring.
        pending_out = []

        def flush_out():
            while pending_out:
                gs, gw, osb, ssb = pending_out.pop()
                nc.sync.dma_start(out[:, gs : gs + gw], osb[:])
                nc.sync.dma_start(skip[:, gs : gs + gw], ssb[:])

        for g in range(NCH):
            gs, gw = CHUNK_STARTS[g], CHUNK_WIDTHS[g]
            for gg in range(g + 1, min(g + PREFETCH + 1, NCH)):
                if xc_tiles[gg] is None:
                    load_chunk(gg)
            xc, cc = xc_tiles[g], cc_tiles[g]
            # let PE observe the chunk DMA sems on a standalone ldweights
            # so no accumulating matmul needs two waits
            nc.tensor.ldweights(xc[:, 0:R])
            nc.tensor.ldweights(cc[:, 0:R])

            osb = opool.tile([R, gw], FP32, tag="osb")
            ssb = opool.tile([S, gw], FP32, tag="ssb")
            for l0 in range(0, gw, NT):
                w = min(NT, gw - l0)
                ztan = zpsum.tile([R, w], FP32, tag="ztan")
                zsig = zpsum.tile([R, w], FP32, tag="zsig")
                for k in range(KW):
                    xs = xc[:, l0 + dilation * k : l0 + dilation * k + w]
                    nc.tensor.matmul(
                        ztan[:], wconv_sb[:, k * R : (k + 1) * R], xs,
                        start=(k == 0), stop=False,
                    )
                nc.tensor.matmul(
                    ztan[:], wcond_sb[:, 0:R], cc[:, l0 : l0 + w],
                    start=False, stop=True,
                )
                for k in range(KW):
                    xs = xc[:, l0 + dilation * k : l0 + dilation * k + w]
                    nc.tensor.matmul(
                        zsig[:], wconv_sb[:, (KW + k) * R : (KW + k + 1) * R], xs,
                        start=(k == 0), stop=False,
                    )
                nc.tensor.matmul(
                    zsig[:], wcond_sb[:, R : 2 * R], cc[:, l0 : l0 + w],
                    start=False, stop=True,
                )

                th = hpool.tile([R, w], BF16, tag="th")
                nc.scalar.activation(th[:], ztan[:], AF.Tanh, bias=zbias_sb[:, 0:1])
                sg = hpool.tile([R, w], BF16, tag="sg")
                nc.scalar.activation(sg[:], zsig[:], AF.Sigmoid, bias=zbias_sb[:, 1:2])
                h = hpool.tile([R, w], BF16, tag="h")
                nc.vector.tensor_mul(h[:], th[:], sg[:])

                for o in range(0, w, NT):
                    po = opsum.tile([R, NT], FP32, tag="po")
                    nc.tensor.matmul(
                        po[:], wos_sb[:, 0:R], h[:, o : o + NT], start=True, stop=True
                    )
                    ps = opsum.tile([S, NT], FP32, tag="ps")
                    nc.tensor.matmul(
                        ps[:], wos_sb[:, R : R + S], h[:, o : o + NT],
                        start=True, stop=True,
                    )
                    nc.vector.tensor_copy(osb[:, l0 + o : l0 + o + NT], po[:])
                    nc.vector.tensor_copy(ssb[:, l0 + o : l0 + o + NT], ps[:])

            flush_out()
            if g == NCH - 1:
                # no input triggers remain behind these: flush immediately
                nc.sync.dma_start(out[:, gs : gs + gw], osb[:])
                nc.sync.dma_start(skip[:, gs : gs + gw], ssb[:])
            else:
                pending_out.append((gs, gw, osb, ssb))

    nc.compile()
    return nc


def _pack_weights(w_conv, w_cond, w_out, w_skip, b_conv, b_cond):
    bf = ml_dtypes.bfloat16
    wconv_p = np.empty((CIN, 2 * KW * R), dtype=bf)
    for k in range(KW):
        wconv_p[:, k * R : (k + 1) * R] = w_conv[0:R, :, k].T.astype(bf)
        wconv_p[:, (KW + k) * R : (KW + k + 1) * R] = w_conv[R : 2 * R, :, k].T.astype(bf)
    wcond_p = np.concatenate(
        [w_cond[0:R, :, 0].T, w_cond[R : 2 * R, :, 0].T], axis=1
    ).astype(bf)
    wos_p = np.concatenate([w_out[:, :, 0].T, w_skip[:, :, 0].T], axis=1).astype(bf)
    zbias_p = np.stack(
        [b_conv[:R] + b_cond[:R], b_conv[R:] + b_cond[R:]], axis=1
    ).astype(np.float32)
    return wconv_p, wcond_p, wos_p, zbias_p


def kernel(**inputs):
    x = np.asarray(inputs["x"], dtype=np.float32)
    cond = np.asarray(inputs["cond"], dtype=np.float32)
    w_conv = np.asarray(inputs["w_conv"], dtype=np.float32)
    b_conv = np.asarray(inputs["b_conv"], dtype=np.float32)
    w_cond = np.asarray(inputs["w_cond"], dtype=np.float32)
    b_cond = np.asarray(inputs["b_cond"], dtype=np.float32)
    w_out = np.asarray(inputs["w_out"], dtype=np.float32)
    b_out = np.asarray(inputs["b_out"], dtype=np.float32)
    w_skip = np.asarray(inputs["w_skip"], dtype=np.float32)
    b_skip = np.asarray(inputs["b_skip"], dtype=np.float32)
    dilation = int(np.asarray(inputs["dilation"]))
    pad = dilation * (KW - 1)

    if dilation not in _built:
        _built[dilation] = _build(dilation)
    nc = _built[dilation]

    wconv_p, wcond_p, wos_p, zbias_p = _pack_weights(
        w_conv, w_cond, w_out, w_skip, b_conv, b_cond
    )
    bf = ml_dtypes.bfloat16
    xb = np.zeros((B, CIN, pad + T), dtype=bf)
    xb[:, :, pad:] = x.astype(bf)
    cb = np.ascontiguousarray(cond.astype(bf))

    in_maps = [
        {
            "x": xb[b],
            "cond": cb[b],
            "wconv": wconv_p,
            "wcond": wcond_p,
            "wos": wos_p,
            "zbias": zbias_p,
        }
        for b in range(B)
    ]
    br = run_bass_kernel_spmd(nc, in_maps, list(range(N_CORES)), trace=_TRACE)
    global _last_results
    _last_results = br
    res = br.results
    output = np.stack([res[b]["out"] for b in range(B)])
    skip = np.stack([res[b]["skip"] for b in range(B)])
    if b_out.any():
        output = output + b_out[None, :, None]
    if b_skip.any():
        skip = skip + b_skip[None, :, None]
    return (output, skip)
